# revision 17
# baseline (speedup 1.0000x reference)
import numpy as np
import ml_dtypes
from contextlib import ExitStack

import concourse.bass as bass
import concourse.bacc as bacc
import concourse.tile as tile
import concourse.mybir as mybir
from concourse.bass import ts, ds

P = 128
NCORES = 8
B_FULL, DIN, DD = 16384, 1024, 2048
BSH = B_FULL // NCORES
CH = 256
NCH = BSH // CH
IT = DD // P
KP = IT // 2
KW = DIN // P
CN = 512
G = 5
BS = 11

F16 = mybir.dt.float16
F32 = mybir.dt.float32
F8 = mybir.dt.float8e4
ADD = mybir.AluOpType.add
SUB = mybir.AluOpType.subtract
MUL = mybir.AluOpType.mult
RELU = mybir.ActivationFunctionType.Relu
DR = mybir.MatmulPerfMode.DoubleRow

_built = {}


def _build(steps: int):
    nc = bacc.Bacc("TRN2", target_bir_lowering=False, debug=False, num_devices=NCORES)

    def inp(name, shape, dt):
        return nc.dram_tensor(name, shape, dt, kind="ExternalInput").ap()

    yTh = inp("yTh", (DIN, BSH), F16)
    Wg_d = inp("Wg", (DIN, DD), F16)
    Sh_d = inp("Sh8", (DD, DD), F8)
    Sl_d = inp("Sl8", (DD, DD), F8)
    Dx_d = inp("Dx16", (DD, DIN), F16)
    nthg_d = inp("nthg", (DD,), F32)
    pthg_d = inp("pthg", (DD,), F32)
    nth_d = inp("nth", (DD,), F32)
    out_d = nc.dram_tensor("out", (BSH, DIN), F32, kind="ExternalOutput").ap()

    NS = max(0, steps - 5)

    with tile.TileContext(nc) as tc, ExitStack() as top:
        thp = top.enter_context(tc.tile_pool(name="thp", bufs=1))
        nthg_t = thp.tile([P, IT], F32)
        pthg_t = thp.tile([P, IT], F32)
        nth_t = thp.tile([P, IT], F32)
        nc.sync.dma_start(nthg_t[:], nthg_d.rearrange("(io p) -> p io", p=P))
        nc.sync.dma_start(pthg_t[:], pthg_d.rearrange("(io p) -> p io", p=P))
        nc.sync.dma_start(nth_t[:], nth_d.rearrange("(io p) -> p io", p=P))

        wpool = top.enter_context(tc.tile_pool(name="wpool", bufs=1))
        Wg_t = wpool.tile([P, KW, DD], F16, name="Wg_t")
        for ko in range(KW):
            nc.sync.dma_start(Wg_t[:, ko, :], Wg_d[ts(ko, P), :])
        Sh_t = wpool.tile([P, KP, 2, DD], F8, name="Sh_t")
        Sl_t = wpool.tile([P, KP, 2, DD], F8, name="Sl_t")
        for kp in range(KP):
            for j in range(2):
                nc.sync.dma_start(Sh_t[:, kp, j, :], Sh_d[ts(2 * kp + j, P), :])
                nc.sync.dma_start(Sl_t[:, kp, j, :], Sl_d[ts(2 * kp + j, P), :])
        Dx_t = wpool.tile([P, IT, DIN], F16, name="Dx_t")
        for io in range(IT):
            nc.sync.dma_start(Dx_t[:, io, :], Dx_d[ts(io, P), :])

        ypool = top.enter_context(tc.tile_pool(name="ypool", bufs=2))
        w0pool = top.enter_context(tc.tile_pool(name="w0pool", bufs=2))
        upool = top.enter_context(tc.tile_pool(name="upool", bufs=4))
        apool = top.enter_context(tc.tile_pool(name="apool", bufs=2))
        psA = top.enter_context(tc.tile_pool(name="psA", bufs=2, space="PSUM"))
        psB = top.enter_context(tc.tile_pool(name="psB", bufs=4, space="PSUM"))
        psC = top.enter_context(tc.tile_pool(name="psC", bufs=2, space="PSUM"))
        wsp = top.enter_context(tc.tile_pool(name="wsp", bufs=2))
        pp = top.enter_context(tc.tile_pool(name="pp", bufs=2))
        qp = top.enter_context(tc.tile_pool(name="qp", bufs=2))
        u16p = top.enter_context(tc.tile_pool(name="u16p", bufs=2))
        stC = top.enter_context(tc.tile_pool(name="stC", bufs=1))

        inv_bs = float(2.0 ** (-BS))
        inv_g = float(2.0 ** (-G))

        MIN = mybir.AluOpType.min
        MAX = mybir.AluOpType.max

        def shrink_split(w_ap, i, uh_n, ul_n):
            bias = nthg_t[:, i:i + 1]
            p_t = pp.tile([P, CH], F16, tag="p")
            q_t = qp.tile([P, CH], F16, tag="q")
            nc.scalar.activation(p_t[:], w_ap, RELU, bias=bias)
            nc.scalar.activation(q_t[:], w_ap, RELU, bias=bias, scale=-1.0)
            u16 = u16p.tile([P, CH], F16, tag="u16")
            nc.vector.tensor_tensor(u16[:], p_t[:], q_t[:], SUB)
            eng = nc.gpsimd if (i % 4 == 3) else nc.vector
            eng.tensor_copy(uh_n[:, i, :], u16[:])
            eng.tensor_tensor(ul_n[:, i, :], u16[:], uh_n[:, i, :], SUB)

        def shrink_fast(w_ap, i, uh_n):
            if i < 10:
                bias = nthg_t[:, i:i + 1]
                p_t = pp.tile([P, CH], F16, tag="p")
                q_t = qp.tile([P, CH], F16, tag="q")
                nc.scalar.activation(p_t[:], w_ap, RELU, bias=bias)
                nc.scalar.activation(q_t[:], w_ap, RELU, bias=bias, scale=-1.0)
                eng = nc.vector if (i % 2 == 0) else nc.gpsimd
                eng.tensor_tensor(uh_n[:, i, :], p_t[:], q_t[:], SUB)
            else:
                eng = nc.vector if (i >= 14) else nc.gpsimd
                c_t = u16p.tile([P, CH], F16, tag="clip")
                eng.tensor_scalar(c_t[:], w_ap, pthg_t[:, i:i + 1],
                                  nthg_t[:, i:i + 1], MIN, MAX)
                eng.tensor_tensor(uh_n[:, i, :], w_ap, c_t[:], SUB)

        def shrink_final(w_ap, i, a16):
            bias = nth_t[:, i:i + 1]
            p_t = pp.tile([P, CH], F16, tag="p")
            q_t = qp.tile([P, CH], F16, tag="q")
            nc.scalar.activation(p_t[:], w_ap, RELU, bias=bias, scale=inv_g)
            nc.scalar.activation(q_t[:], w_ap, RELU, bias=bias, scale=-inv_g)
            nc.vector.tensor_tensor(a16[:, i, :], p_t[:], q_t[:], SUB)

        need_ul0 = 1 > NS and steps >= 1

        def emit_A(c, st):
            cs = ds(c * CH, CH)
            yh_t = ypool.tile([P, KW, CH], F16, tag="yh")
            for ko in range(KW):
                nc.sync.dma_start(yh_t[:, ko, :], yTh[ts(ko, P), cs])
            st["w0"] = w0_t = w0pool.tile([P, IT, CH], F16, tag="w0", name="w0_t")
            st["uh"] = uh_c = upool.tile([P, IT, CH], F8, tag="uh", name="uh_c")
            st["ul"] = ul_c = (upool.tile([P, IT, CH], F8, tag="ul", name="ul_c")
                              if need_ul0 else None)
            for i in range(IT):
                ps = psA.tile([P, CH], F32, tag="psA")
                for ko in range(KW):
                    nc.tensor.matmul(ps[:], Wg_t[:, ko, ts(i, P)], yh_t[:, ko, :],
                                     start=(ko == 0), stop=(ko == KW - 1))
                nc.vector.tensor_copy(w0_t[:, i, :], ps[:])
                if need_ul0:
                    shrink_split(w0_t[:, i, :], i, uh_c, ul_c)
                else:
                    shrink_fast(w0_t[:, i, :], i, uh_c)

        def emit_B_step(t, st):
            last = t == steps
            three = t > NS
            uh_c, ul_c, w0_t = st["uh"], st["ul"], st["w0"]
            if last:
                st["a16"] = a16 = apool.tile([P, IT, CH], F16, tag="a16", name="a16")
            else:
                need_ul_n = (t + 1) > NS
                uh_n = upool.tile([P, IT, CH], F8, tag="uh", name="uh_n")
                ul_n = (upool.tile([P, IT, CH], F8, tag="ul", name="ul_n")
                        if need_ul_n else None)
            for i in range(IT):
                ps = psB.tile([P, CH], F32, tag="psB")
                n_mm = 3 * KP if three else KP
                k = 0

                def mm(S_t, u_t, kp):
                    nonlocal k
                    nc.tensor.matmul(ps[:], S_t[:, kp, :, ts(i, P)],
                                     u_t[:, ds(2 * kp, 2), :],
                                     start=(k == 0), stop=(k == n_mm - 1),
                                     perf_mode=DR)
                    k += 1

                if three:
                    for kp in range(KP - 1):
                        mm(Sh_t, uh_c, kp)
                    for kp in range(KP - 1):
                        mm(Sl_t, uh_c, kp)
                    mm(Sh_t, uh_c, KP - 1)
                    mm(Sl_t, uh_c, KP - 1)
                    for kp in range(KP):
                        mm(Sh_t, ul_c, kp)
                else:
                    for kp in range(KP):
                        mm(Sh_t, uh_c, kp)
                w_t = wsp.tile([P, CH], F16, tag="w")
                nc.vector.scalar_tensor_tensor(w_t[:], ps[:], inv_bs, w0_t[:, i, :],
                                               MUL, ADD)
                if last:
                    shrink_final(w_t[:], i, a16)
                elif need_ul_n:
                    shrink_split(w_t[:], i, uh_n, ul_n)
                else:
                    shrink_fast(w_t[:], i, uh_n)
            if not last:
                st["uh"], st["ul"] = uh_n, (ul_n if need_ul_n else None)

        def emit_C(c, st):
            a16 = st["a16"]
            for bt in range(CH // P):
                for dn in range(DIN // CN):
                    ps = psC.tile([P, CN], F32, tag="psC")
                    for io in range(IT):
                        nc.tensor.matmul(ps[:], a16[:, io, ts(bt, P)],
                                         Dx_t[:, io, ts(dn, CN)],
                                         start=(io == 0), stop=(io == IT - 1))
                    st_t = stC.tile([P, CN], F32, tag="stC")
                    nc.vector.tensor_copy(st_t[:], ps[:])
                    nc.sync.dma_start(out_d[ds(c * CH + bt * P, P), ts(dn, CN)],
                                      st_t[:])

        for cp in range(0, NCH, 2):
            pair = [cp, cp + 1] if cp + 1 < NCH else [cp]
            states = {c: {} for c in pair}
            for c in pair:
                emit_A(c, states[c])
            for t in range(1, steps + 1):
                for c in pair:
                    emit_B_step(t, states[c])
            for c in pair:
                emit_C(c, states[c])

    nc.compile()
    return nc


def _prep_in_maps(y, W, Theta, S, Dx):
    y = np.ascontiguousarray(np.asarray(y, dtype=np.float32))
    W = np.asarray(W, dtype=np.float32)
    Theta = np.asarray(Theta, dtype=np.float32)
    S = np.asarray(S, dtype=np.float32)
    Dx = np.asarray(Dx, dtype=np.float32)
    assert y.shape == (B_FULL, DIN) and W.shape == (DIN, DD)
    assert S.shape == (DD, DD) and Dx.shape == (DD, DIN)

    E4 = ml_dtypes.float8_e4m3
    th = np.maximum(Theta, 0.0).astype(np.float32) + np.float32(1e-7)
    Wg = (W * np.float32(2.0 ** G)).astype(np.float16)
    Ss = (S * np.float32(2.0 ** BS)).astype(np.float32)
    Sh8 = Ss.astype(E4)
    Sl8 = (Ss - Sh8.astype(np.float32)).astype(E4)
    Dx16 = Dx.astype(np.float16)
    nthg = (-(th * np.float32(2.0 ** G))).astype(np.float32)
    pthg = (th * np.float32(2.0 ** G)).astype(np.float32)
    nth = (-th).astype(np.float32)
    yT = np.ascontiguousarray(y.T)
    yTh_f = yT.astype(np.float16)

    shared = dict(Wg=Wg, Sh8=Sh8, Sl8=Sl8, Dx16=Dx16, nthg=nthg, pthg=pthg, nth=nth)
    in_maps = []
    for c in range(NCORES):
        sl = slice(c * BSH, (c + 1) * BSH)
        in_maps.append(dict(shared, yTh=np.ascontiguousarray(yTh_f[:, sl])))
    return in_maps


_sharded_cache = {}


def _get_sharded(steps: int):
    if steps in _sharded_cache:
        return _sharded_cache[steps]
    import jax
    from jax.experimental.shard_map import shard_map
    from jax.sharding import Mesh, PartitionSpec
    from concourse import bass2jax

    if steps not in _built:
        _built[steps] = _build(steps)
    nc = _built[steps]
    bass2jax.install_neuronx_cc_hook()
    assert nc.dbg_addr is None
    partition_name = nc.partition_id_tensor.name if nc.partition_id_tensor else None

    in_names, out_names, out_avals, zero_shapes = [], [], [], []
    for alloc in nc.m.functions[0].allocations:
        if not isinstance(alloc, mybir.MemoryLocationSet):
            continue
        name = alloc.memorylocations[0].name
        if alloc.kind == "ExternalInput":
            if name != partition_name:
                in_names.append(name)
        elif alloc.kind == "ExternalOutput":
            out_names.append(name)
            shape = tuple(alloc.tensor_shape)
            dtype = mybir.dt.np(alloc.dtype)
            out_avals.append(jax.core.ShapedArray(shape, dtype))
            zero_shapes.append((shape, dtype))
    n_params = len(in_names)
    n_outs = len(out_names)
    all_in_names = in_names + out_names
    if partition_name is not None:
        all_in_names.append(partition_name)

    def _body(*args):
        operands = list(args)
        if partition_name is not None:
            operands.append(bass2jax.partition_id_tensor())
        outs = bass2jax._bass_exec_p.bind(
            *operands,
            out_avals=tuple(out_avals),
            in_names=tuple(all_in_names),
            out_names=tuple(out_names),
            lowering_input_output_aliases=(),
            sim_require_finite=True,
            sim_require_nnan=True,
            nc=nc,
        )
        return tuple(outs)

    devices = jax.devices()[:NCORES]
    mesh = Mesh(np.asarray(devices), ("core",))
    donate = tuple(range(n_params, n_params + n_outs))
    sharded = jax.jit(
        shard_map(_body, mesh=mesh,
                  in_specs=(PartitionSpec("core"),) * (n_params + n_outs),
                  out_specs=(PartitionSpec("core"),) * n_outs,
                  check_rep=False),
        donate_argnums=donate, keep_unused=True)
    entry = dict(sharded=sharded, in_names=in_names, out_names=out_names,
                 zero_shapes=zero_shapes, mesh=mesh, n_params=n_params)
    _sharded_cache[steps] = entry
    return entry


def _concat_inputs(entry, in_maps):
    return [np.concatenate([np.asarray(in_maps[c][n]) for c in range(NCORES)], axis=0)
            for n in entry["in_names"]]


def _run(entry, concat_in):
    zeros = [np.zeros((NCORES * s[0], *s[1:]), d) for s, d in entry["zero_shapes"]]
    out_arrs = entry["sharded"](*concat_in, *zeros)
    return out_arrs


def kernel(y, W, Theta, S, Dx, unroll_steps):
    steps = int(unroll_steps)
    entry = _get_sharded(steps)
    in_maps = _prep_in_maps(y, W, Theta, S, Dx)
    out_arrs = _run(entry, _concat_inputs(entry, in_maps))
    idx = entry["out_names"].index("out")
    return np.ascontiguousarray(np.asarray(out_arrs[idx]))


def time_kernel(np_inputs, iters=6):
    import jax
    from jax.sharding import NamedSharding, PartitionSpec
    steps = int(np_inputs["unroll_steps"])
    entry = _get_sharded(steps)
    in_maps = _prep_in_maps(np_inputs["y"], np_inputs["W"], np_inputs["Theta"],
                            np_inputs["S"], np_inputs["Dx"])
    concat_in = _concat_inputs(entry, in_maps)
    sh = NamedSharding(entry["mesh"], PartitionSpec("core"))
    dev_in = [jax.device_put(a, sh) for a in concat_in]
    import time as _time
    times = []
    for it in range(iters):
        zeros = [jax.device_put(np.zeros((NCORES * s[0], *s[1:]), d), sh)
                 for s, d in entry["zero_shapes"]]
        for z in zeros:
            z.block_until_ready()
        t0 = _time.perf_counter()
        outs = entry["sharded"](*dev_in, *zeros)
        for o in outs:
            o.block_until_ready()
        times.append(_time.perf_counter() - t0)
    best = min(times[1:]) if len(times) > 1 else times[0]
    print("  per-iter times (ms):", [f"{t*1e3:.1f}" for t in times])
    return best * 1e9


if __name__ == "__main__":
    rng = np.random.default_rng(0)
    inputs = dict(
        y=rng.standard_normal((B_FULL, DIN), dtype=np.float32),
        W=(rng.standard_normal((DIN, DD)) * 0.02).astype(np.float32),
        Theta=rng.random(DD, dtype=np.float32),
        S=(rng.standard_normal((DD, DD)) * 0.02).astype(np.float32),
        Dx=(rng.standard_normal((DD, DIN)) * 0.02).astype(np.float32),
        unroll_steps=16,
    )
    out = kernel(**inputs)
    print("out", out.shape, out.dtype, np.abs(out).max())


# revision 18
# speedup vs baseline: 1.1615x; 1.1615x over previous
import numpy as np
import ml_dtypes
from contextlib import ExitStack

import concourse.bass as bass
import concourse.bacc as bacc
import concourse.tile as tile
import concourse.mybir as mybir
from concourse.bass import ts, ds

P = 128
NCORES = 8
B_FULL, DIN, DD = 16384, 1024, 2048
BSH = B_FULL // NCORES
CH = 256
NCH = BSH // CH
IT = DD // P
KP = IT // 2
KW = DIN // P
CN = 512
G = 5
BS = 11

F16 = mybir.dt.float16
F32 = mybir.dt.float32
F8 = mybir.dt.float8e4
ADD = mybir.AluOpType.add
SUB = mybir.AluOpType.subtract
MUL = mybir.AluOpType.mult
RELU = mybir.ActivationFunctionType.Relu
DR = mybir.MatmulPerfMode.DoubleRow

_built = {}


def _build(steps: int):
    nc = bacc.Bacc("TRN2", target_bir_lowering=False, debug=False, num_devices=NCORES)

    def inp(name, shape, dt):
        return nc.dram_tensor(name, shape, dt, kind="ExternalInput").ap()

    yTh = inp("yTh", (DIN, BSH), F16)
    Wg_d = inp("Wg", (DIN, DD), F16)
    Sh_d = inp("Sh8", (DD, DD), F8)
    Sl_d = inp("Sl8", (DD, DD), F8)
    Dx_d = inp("Dx16", (DD, DIN), F16)
    nthg_d = inp("nthg", (DD,), F32)
    pthg_d = inp("pthg", (DD,), F32)
    nth_d = inp("nth", (DD,), F32)
    out_d = nc.dram_tensor("out", (BSH, DIN), F32, kind="ExternalOutput").ap()

    NS = max(0, steps - 5)

    with tile.TileContext(nc) as tc, ExitStack() as top:
        thp = top.enter_context(tc.tile_pool(name="thp", bufs=1))
        nthg_t = thp.tile([P, IT], F32)
        pthg_t = thp.tile([P, IT], F32)
        nth_t = thp.tile([P, IT], F32)
        nc.sync.dma_start(nthg_t[:], nthg_d.rearrange("(io p) -> p io", p=P))
        nc.sync.dma_start(pthg_t[:], pthg_d.rearrange("(io p) -> p io", p=P))
        nc.sync.dma_start(nth_t[:], nth_d.rearrange("(io p) -> p io", p=P))

        wpool = top.enter_context(tc.tile_pool(name="wpool", bufs=1))
        Wg_t = wpool.tile([P, KW, DD], F16, name="Wg_t")
        for ko in range(KW):
            nc.sync.dma_start(Wg_t[:, ko, :], Wg_d[ts(ko, P), :])
        Sh_t = wpool.tile([P, KP, 2, DD], F8, name="Sh_t")
        Sl_t = wpool.tile([P, KP, 2, DD], F8, name="Sl_t")
        for kp in range(KP):
            for j in range(2):
                nc.sync.dma_start(Sh_t[:, kp, j, :], Sh_d[ts(2 * kp + j, P), :])
                nc.sync.dma_start(Sl_t[:, kp, j, :], Sl_d[ts(2 * kp + j, P), :])
        Dx_t = wpool.tile([P, IT, DIN], F16, name="Dx_t")
        for io in range(IT):
            nc.sync.dma_start(Dx_t[:, io, :], Dx_d[ts(io, P), :])

        ypool = top.enter_context(tc.tile_pool(name="ypool", bufs=2))
        w0pool = top.enter_context(tc.tile_pool(name="w0pool", bufs=2))
        upool = top.enter_context(tc.tile_pool(name="upool", bufs=4))
        apool = top.enter_context(tc.tile_pool(name="apool", bufs=1))
        psA = top.enter_context(tc.tile_pool(name="psA", bufs=2, space="PSUM"))
        psB = top.enter_context(tc.tile_pool(name="psB", bufs=4, space="PSUM"))
        psC = top.enter_context(tc.tile_pool(name="psC", bufs=2, space="PSUM"))
        wsp = top.enter_context(tc.tile_pool(name="wsp", bufs=4))
        pp = top.enter_context(tc.tile_pool(name="pp", bufs=4))
        qp = top.enter_context(tc.tile_pool(name="qp", bufs=4))
        u16p = top.enter_context(tc.tile_pool(name="u16p", bufs=4))
        stC = top.enter_context(tc.tile_pool(name="stC", bufs=1))

        inv_bs = float(2.0 ** (-BS))
        inv_g = float(2.0 ** (-G))

        MIN = mybir.AluOpType.min
        MAX = mybir.AluOpType.max

        def shrink_split(w_ap, i, uh_n, ul_n):
            bias = nthg_t[:, i:i + 1]
            p_t = pp.tile([P, CH], F16, tag="p")
            q_t = qp.tile([P, CH], F16, tag="q")
            nc.scalar.activation(p_t[:], w_ap, RELU, bias=bias)
            nc.scalar.activation(q_t[:], w_ap, RELU, bias=bias, scale=-1.0)
            u16 = u16p.tile([P, CH], F16, tag="u16")
            nc.vector.tensor_tensor(u16[:], p_t[:], q_t[:], SUB)
            eng = nc.gpsimd if (i % 4 == 3) else nc.vector
            eng.tensor_copy(uh_n[:, i, :], u16[:])
            eng.tensor_tensor(ul_n[:, i, :], u16[:], uh_n[:, i, :], SUB)

        def shrink_fast(w_ap, i, uh_n):
            if i < 10:
                bias = nthg_t[:, i:i + 1]
                p_t = pp.tile([P, CH], F16, tag="p")
                q_t = qp.tile([P, CH], F16, tag="q")
                nc.scalar.activation(p_t[:], w_ap, RELU, bias=bias)
                nc.scalar.activation(q_t[:], w_ap, RELU, bias=bias, scale=-1.0)
                eng = nc.vector if (i % 2 == 0) else nc.gpsimd
                eng.tensor_tensor(uh_n[:, i, :], p_t[:], q_t[:], SUB)
            else:
                eng = nc.vector if (i >= 14) else nc.gpsimd
                c_t = u16p.tile([P, CH], F16, tag="clip")
                eng.tensor_scalar(c_t[:], w_ap, pthg_t[:, i:i + 1],
                                  nthg_t[:, i:i + 1], MIN, MAX)
                eng.tensor_tensor(uh_n[:, i, :], w_ap, c_t[:], SUB)

        def shrink_final(w_ap, i, a16):
            bias = nth_t[:, i:i + 1]
            p_t = pp.tile([P, CH], F16, tag="p")
            q_t = qp.tile([P, CH], F16, tag="q")
            nc.scalar.activation(p_t[:], w_ap, RELU, bias=bias, scale=inv_g)
            nc.scalar.activation(q_t[:], w_ap, RELU, bias=bias, scale=-inv_g)
            nc.vector.tensor_tensor(a16[:, i, :], p_t[:], q_t[:], SUB)

        need_ul0 = 1 > NS and steps >= 1

        def emit_A(c, st):
            cs = ds(c * CH, CH)
            yh_t = ypool.tile([P, KW, CH], F16, tag="yh")
            for ko in range(KW):
                nc.sync.dma_start(yh_t[:, ko, :], yTh[ts(ko, P), cs])
            st["w0"] = w0_t = w0pool.tile([P, IT, CH], F16, tag="w0", name="w0_t")
            st["uh"] = uh_c = upool.tile([P, IT, CH], F8, tag="uh", name="uh_c")
            st["ul"] = ul_c = (upool.tile([P, IT, CH], F8, tag="ul", name="ul_c")
                              if need_ul0 else None)
            for i in range(IT):
                ps = psA.tile([P, CH], F32, tag="psA")
                for ko in range(KW):
                    nc.tensor.matmul(ps[:], Wg_t[:, ko, ts(i, P)], yh_t[:, ko, :],
                                     start=(ko == 0), stop=(ko == KW - 1))
                nc.vector.tensor_copy(w0_t[:, i, :], ps[:])
                if need_ul0:
                    shrink_split(w0_t[:, i, :], i, uh_c, ul_c)
                else:
                    shrink_fast(w0_t[:, i, :], i, uh_c)

        def emit_B_step(t, st):
            last = t == steps
            three = t > NS
            uh_c, ul_c, w0_t = st["uh"], st["ul"], st["w0"]
            if last:
                st["a16"] = a16 = apool.tile([P, IT, CH], F16, tag="a16", name="a16")
            else:
                need_ul_n = (t + 1) > NS
                uh_n = upool.tile([P, IT, CH], F8, tag="uh", name="uh_n")
                ul_n = (upool.tile([P, IT, CH], F8, tag="ul", name="ul_n")
                        if need_ul_n else None)
            for i in range(IT):
                ps = psB.tile([P, CH], F32, tag="psB")
                n_mm = 3 * KP if three else KP
                k = 0

                def mm(S_t, u_t, kp):
                    nonlocal k
                    nc.tensor.matmul(ps[:], S_t[:, kp, :, ts(i, P)],
                                     u_t[:, ds(2 * kp, 2), :],
                                     start=(k == 0), stop=(k == n_mm - 1),
                                     perf_mode=DR)
                    k += 1

                if three:
                    for kp in range(KP - 1):
                        mm(Sh_t, uh_c, kp)
                    for kp in range(KP - 1):
                        mm(Sl_t, uh_c, kp)
                    mm(Sh_t, uh_c, KP - 1)
                    mm(Sl_t, uh_c, KP - 1)
                    for kp in range(KP):
                        mm(Sh_t, ul_c, kp)
                else:
                    for kp in range(KP):
                        mm(Sh_t, uh_c, kp)
                w_t = wsp.tile([P, CH], F16, tag="w")
                nc.vector.scalar_tensor_tensor(w_t[:], ps[:], inv_bs, w0_t[:, i, :],
                                               MUL, ADD)
                if last:
                    shrink_final(w_t[:], i, a16)
                elif need_ul_n:
                    shrink_split(w_t[:], i, uh_n, ul_n)
                else:
                    shrink_fast(w_t[:], i, uh_n)
            if not last:
                st["uh"], st["ul"] = uh_n, (ul_n if need_ul_n else None)

        def emit_C(c, st):
            a16 = st["a16"]
            for bt in range(CH // P):
                for dn in range(DIN // CN):
                    ps = psC.tile([P, CN], F32, tag="psC")
                    for io in range(IT):
                        nc.tensor.matmul(ps[:], a16[:, io, ts(bt, P)],
                                         Dx_t[:, io, ts(dn, CN)],
                                         start=(io == 0), stop=(io == IT - 1))
                    st_t = stC.tile([P, CN], F32, tag="stC")
                    nc.vector.tensor_copy(st_t[:], ps[:])
                    nc.sync.dma_start(out_d[ds(c * CH + bt * P, P), ts(dn, CN)],
                                      st_t[:])

        for cp in range(0, NCH, 2):
            pair = [cp, cp + 1] if cp + 1 < NCH else [cp]
            states = {c: {} for c in pair}
            for c in pair:
                emit_A(c, states[c])
            for t in range(1, steps + 1):
                for c in pair:
                    emit_B_step(t, states[c])
            for c in pair:
                emit_C(c, states[c])

    nc.compile()
    return nc


def _prep_in_maps(y, W, Theta, S, Dx):
    y = np.ascontiguousarray(np.asarray(y, dtype=np.float32))
    W = np.asarray(W, dtype=np.float32)
    Theta = np.asarray(Theta, dtype=np.float32)
    S = np.asarray(S, dtype=np.float32)
    Dx = np.asarray(Dx, dtype=np.float32)
    assert y.shape == (B_FULL, DIN) and W.shape == (DIN, DD)
    assert S.shape == (DD, DD) and Dx.shape == (DD, DIN)

    E4 = ml_dtypes.float8_e4m3
    th = np.maximum(Theta, 0.0).astype(np.float32) + np.float32(1e-7)
    Wg = (W * np.float32(2.0 ** G)).astype(np.float16)
    Ss = (S * np.float32(2.0 ** BS)).astype(np.float32)
    Sh8 = Ss.astype(E4)
    Sl8 = (Ss - Sh8.astype(np.float32)).astype(E4)
    Dx16 = Dx.astype(np.float16)
    nthg = (-(th * np.float32(2.0 ** G))).astype(np.float32)
    pthg = (th * np.float32(2.0 ** G)).astype(np.float32)
    nth = (-th).astype(np.float32)
    yT = np.ascontiguousarray(y.T)
    yTh_f = yT.astype(np.float16)

    shared = dict(Wg=Wg, Sh8=Sh8, Sl8=Sl8, Dx16=Dx16, nthg=nthg, pthg=pthg, nth=nth)
    in_maps = []
    for c in range(NCORES):
        sl = slice(c * BSH, (c + 1) * BSH)
        in_maps.append(dict(shared, yTh=np.ascontiguousarray(yTh_f[:, sl])))
    return in_maps


_sharded_cache = {}


def _get_sharded(steps: int):
    if steps in _sharded_cache:
        return _sharded_cache[steps]
    import jax
    from jax.experimental.shard_map import shard_map
    from jax.sharding import Mesh, PartitionSpec
    from concourse import bass2jax

    if steps not in _built:
        _built[steps] = _build(steps)
    nc = _built[steps]
    bass2jax.install_neuronx_cc_hook()
    assert nc.dbg_addr is None
    partition_name = nc.partition_id_tensor.name if nc.partition_id_tensor else None

    in_names, out_names, out_avals, zero_shapes = [], [], [], []
    for alloc in nc.m.functions[0].allocations:
        if not isinstance(alloc, mybir.MemoryLocationSet):
            continue
        name = alloc.memorylocations[0].name
        if alloc.kind == "ExternalInput":
            if name != partition_name:
                in_names.append(name)
        elif alloc.kind == "ExternalOutput":
            out_names.append(name)
            shape = tuple(alloc.tensor_shape)
            dtype = mybir.dt.np(alloc.dtype)
            out_avals.append(jax.core.ShapedArray(shape, dtype))
            zero_shapes.append((shape, dtype))
    n_params = len(in_names)
    n_outs = len(out_names)
    all_in_names = in_names + out_names
    if partition_name is not None:
        all_in_names.append(partition_name)

    def _body(*args):
        operands = list(args)
        if partition_name is not None:
            operands.append(bass2jax.partition_id_tensor())
        outs = bass2jax._bass_exec_p.bind(
            *operands,
            out_avals=tuple(out_avals),
            in_names=tuple(all_in_names),
            out_names=tuple(out_names),
            lowering_input_output_aliases=(),
            sim_require_finite=True,
            sim_require_nnan=True,
            nc=nc,
        )
        return tuple(outs)

    devices = jax.devices()[:NCORES]
    mesh = Mesh(np.asarray(devices), ("core",))
    donate = tuple(range(n_params, n_params + n_outs))
    sharded = jax.jit(
        shard_map(_body, mesh=mesh,
                  in_specs=(PartitionSpec("core"),) * (n_params + n_outs),
                  out_specs=(PartitionSpec("core"),) * n_outs,
                  check_rep=False),
        donate_argnums=donate, keep_unused=True)
    entry = dict(sharded=sharded, in_names=in_names, out_names=out_names,
                 zero_shapes=zero_shapes, mesh=mesh, n_params=n_params)
    _sharded_cache[steps] = entry
    return entry


def _concat_inputs(entry, in_maps):
    return [np.concatenate([np.asarray(in_maps[c][n]) for c in range(NCORES)], axis=0)
            for n in entry["in_names"]]


def _run(entry, concat_in):
    zeros = [np.zeros((NCORES * s[0], *s[1:]), d) for s, d in entry["zero_shapes"]]
    out_arrs = entry["sharded"](*concat_in, *zeros)
    return out_arrs


def kernel(y, W, Theta, S, Dx, unroll_steps):
    steps = int(unroll_steps)
    entry = _get_sharded(steps)
    in_maps = _prep_in_maps(y, W, Theta, S, Dx)
    out_arrs = _run(entry, _concat_inputs(entry, in_maps))
    idx = entry["out_names"].index("out")
    return np.ascontiguousarray(np.asarray(out_arrs[idx]))


def time_kernel(np_inputs, iters=6):
    import jax
    from jax.sharding import NamedSharding, PartitionSpec
    steps = int(np_inputs["unroll_steps"])
    entry = _get_sharded(steps)
    in_maps = _prep_in_maps(np_inputs["y"], np_inputs["W"], np_inputs["Theta"],
                            np_inputs["S"], np_inputs["Dx"])
    concat_in = _concat_inputs(entry, in_maps)
    sh = NamedSharding(entry["mesh"], PartitionSpec("core"))
    dev_in = [jax.device_put(a, sh) for a in concat_in]
    import time as _time
    times = []
    for it in range(iters):
        zeros = [jax.device_put(np.zeros((NCORES * s[0], *s[1:]), d), sh)
                 for s, d in entry["zero_shapes"]]
        for z in zeros:
            z.block_until_ready()
        t0 = _time.perf_counter()
        outs = entry["sharded"](*dev_in, *zeros)
        for o in outs:
            o.block_until_ready()
        times.append(_time.perf_counter() - t0)
    best = min(times[1:]) if len(times) > 1 else times[0]
    print("  per-iter times (ms):", [f"{t*1e3:.1f}" for t in times])
    return best * 1e9


if __name__ == "__main__":
    rng = np.random.default_rng(0)
    inputs = dict(
        y=rng.standard_normal((B_FULL, DIN), dtype=np.float32),
        W=(rng.standard_normal((DIN, DD)) * 0.02).astype(np.float32),
        Theta=rng.random(DD, dtype=np.float32),
        S=(rng.standard_normal((DD, DD)) * 0.02).astype(np.float32),
        Dx=(rng.standard_normal((DD, DIN)) * 0.02).astype(np.float32),
        unroll_steps=16,
    )
    out = kernel(**inputs)
    print("out", out.shape, out.dtype, np.abs(out).max())
